# revision 64
# baseline (speedup 1.0000x reference)
"""GQA prefill kernel for 8 Trainium2 NeuronCores.

Problem: B=2, T=2048, C=2048, H=32 q-heads, HKV=8 kv-heads, DH=64,
causal attention with RoPE, torch-Linear-style projections.

Sharding: core = b*4 + g over (batch b in 0..1, head-group g in 0..3).
Each core owns 8 q-heads / 2 kv-heads of one batch element:
  - Wq column-shard   -> qT   [512, T]  (features on partitions)
  - Wkv column-shard  -> kT,vT[128, T]
  - Wo row-shard      -> partial output [T, C]; host sums 4 partials/batch.

v2 design notes (vs v1 baseline):
  - Everything bf16 (x, weights, q/k/v, probs, attn-out); PSUM stays f32.
    Matmul cost on TRN2 is per moving row regardless of dtype; bf16
    halves DMA bytes and DVE elementwise cost (2x packed mode).
  - rotate_half via a signed 128x128 permutation matmul on the PE, sign
    folded into the matrix so the sin table is unsigned.
  - Fine-grained causality at 128-col k-tiles. Diagonal 128x128 tiles
    batch 8-up into one [128,1024] PSUM tile (one mask-add + one exp);
    off-diagonal score segments from multiple k-tiles pack into shared
    [128,1024] PSUM tiles, one exp per 1024 columns (pending-zero bank
    semantics let packed segments share banks with one start=True).
  - attn@V in natural layout: probs slices are the stationary operand,
    v_aug [128k, 65] moves (65-row cost vs 512), accumulating [128q,65]
    per q-subtile with the denominator in column 64. One strided
    reciprocal + one 3D broadcast multiply normalizes all 8 subtiles
    into a_buf; XBAR DMA transposes (SP queue) produce aT for phase 3.
  - Cross-head software pipelining keeps PE >= 1 head ahead of the
    ACT-bound exp stream; the first half of the output projection is
    interleaved into attention half 1; weight/x loads are split across
    the SP HWDGE and gpsimd SWDGE queues in consumption order.
"""

import sys

sys.path.insert(0, "/opt/trn_rl_repo")

import numpy as np
import ml_dtypes

import concourse.bass as bass
import concourse.tile as tile
from concourse import bacc
from concourse import mybir
from concourse import bass_utils
from concourse.masks import make_identity

F32 = mybir.dt.float32
F32R = mybir.dt.float32r
BF16 = mybir.dt.bfloat16
AF = mybir.ActivationFunctionType
ALU = mybir.AluOpType

B, T, C, DH = 2, 2048, 2048, 64
NCORE = 8
NEG = -1.0e30


def _r(ap):
    return ap.bitcast(F32R)


def _phase1(tc, cst, io_consts, after_j1=None):
    """Projections + RoPE + v transpose. Fills qT, kT, v_aug.

    Software-pipelined: PE-side RoPE work (rotate-half permutation matmuls,
    v transposes) for one wave is emitted after the NEXT wave's projection
    matmuls, so the PE never waits on the PSUM->SBUF activation copies.
    """
    nc = tc.nc
    (xT, wqT, wq, wkv, cos_sb, sin_sb, perm_sb, ident, qT, kT, v_aug) = io_consts

    with tc.tile_pool(name="p1xt", bufs=18) as xp, \
         tc.tile_pool(name="p1raw", bufs=7) as rawp, \
         tc.tile_pool(name="p1tmp", bufs=3) as tmpp, \
         tc.tile_pool(name="p1sh", bufs=1, space="PSUM") as pssh, \
         tc.tile_pool(name="p1tr", bufs=1, space="PSUM") as pstr, \
         tc.tile_pool(name="p1ps", bufs=4, space="PSUM") as psproj:

        def emit_wave(j, wave, xts):
            """Projection matmuls for 3 feature groups + PSUM->SBUF copies."""
            gs = (0, 1, 2) if wave == 0 else (3, 4, 5)
            accs = {}
            for g in gs:
                accs[g] = psproj.tile([128, 512], F32, tag="proj",
                                      name=f"acc{j}_{g}")
            for c in range(16):
                for g in gs:
                    if g < 4:
                        lh = wq[:, c * 512 + g * 128: c * 512 + (g + 1) * 128]
                    elif g == 4:
                        lh = wkv[:, c * 256: c * 256 + 128]
                    else:
                        lh = wkv[:, c * 256 + 128: c * 256 + 256]
                    nc.tensor.matmul(accs[g][:], lhsT=lh, rhs=xts[c][:],
                                     start=(c == 0), stop=(c == 15))
            raws = {}
            for g in gs:
                raw = rawp.tile([128, 512], BF16, tag="raw",
                                name=f"raw{j}_{g}")
                nc.scalar.copy(raw[:], accs[g][:])
                raws[g] = raw
            return raws

        def emit_rope(j, raws):
            """PE rotate-half / v-transpose + DVE combines for a wave.

            All shp-PSUM-releasing tensor_muls run first on the DVE so the
            next wave's shift matmuls never block on PSUM buffers.
            """
            jc = slice(j * 512, (j + 1) * 512)
            for g, raw in raws.items():
                if g < 5:
                    shp = pssh.tile([128, 512], F32, tag="sh",
                                    name=f"sh{j}_{g}")
                    nc.tensor.matmul(shp[:], lhsT=perm_sb[:], rhs=raw[:],
                                     start=True, stop=True)
                    tmp = tmpp.tile([128, 512], BF16, tag="rt",
                                    name=f"rt{j}_{g}")
                    nc.vector.tensor_mul(tmp[:], shp[:], sin_sb[:, jc])
                    dst = (qT[:, g * 2048 + j * 512: g * 2048 + (j + 1) * 512]
                           if g < 4 else kT[:, jc])
                    nc.vector.tensor_mul(dst, raw[:], cos_sb[:, jc])
                    nc.vector.tensor_add(dst, dst, tmp[:])
                else:
                    for tt in range(4):
                        ptr = pstr.tile([128, 128], BF16, tag="tr",
                                        name=f"tr{j}_{tt}")
                        nc.tensor.transpose(ptr[:],
                                            raw[:, tt * 128:(tt + 1) * 128],
                                            ident[:])
                        gt = j * 4 + tt
                        nc.vector.tensor_copy(
                            v_aug[:, gt * 65: gt * 65 + 64], ptr[:, 0:64])
                        nc.vector.tensor_copy(
                            v_aug[:, 1040 + gt * 65: 1040 + gt * 65 + 64],
                            ptr[:, 64:128])

        pending = None
        for j in range(4):
            jc = slice(j * 512, (j + 1) * 512)
            if pending is not None and pending[0] == 1 and after_j1:
                emit_rope(*pending)
                pending = None
                after_j1()
            xts = []
            for c in range(16):
                xt = xp.tile([128, 512], BF16, tag="xt", name=f"xt{j}_{c}")
                nc.sync.dma_start(out=xt[:],
                                  in_=xT[c * 128:(c + 1) * 128, jc])
                xts.append(xt)
                if j == 0:
                    nc.sync.dma_start(out=wq[:, c * 512:(c + 1) * 512],
                                      in_=wqT[c * 128:(c + 1) * 128, :])
            raws0 = emit_wave(j, 0, xts)
            if pending is not None:
                emit_rope(*pending)
            raws1 = emit_wave(j, 1, xts)
            emit_rope(j, raws0)
            pending = (j, raws1)
        emit_rope(*pending)


def _attn_half(tc, jj, consts, a_buf, extra_work=None, early_prd=None,
               fresh_prp=None, prp_ext=None, early_segs=None):
    """Attention for q-columns [jj*1024, (jj+1)*1024) of all 8 heads.

    ``extra_work``: PE-heavy emitters interleaved between heads (used to
    overlap the first half of the output projection with this ACT-bound
    phase). Costs one psav buffer (PSUM bank budget).
    """
    nc = tc.nc
    (qT, kT, v_aug, mask_sb, aT) = consts
    q0 = jj * 1024
    extra = list(extra_work) if extra_work else []
    fresh_left = [0]

    # half-wide packed-chunk plan: 128-aligned score chunks from all heads
    # stream through shared [128,1024] PSUM tiles (no flush at head ends)
    early_heads = set()
    if early_segs:
        early_heads = {k[0] for k in early_segs}
    all_chunks = []  # (hq, i, gstart_global, width)
    for phq in range(8):
        if phq in early_heads:
            continue
        fill = sum(ch[3] for ch in all_chunks) % 1024
        for i in range(8 * (jj + 1)):
            gstart = max(q0, 128 * (i + 1))
            W = q0 + 1024 - gstart
            off = 0
            while off < W:
                c = min(W - off, 512 - (fill % 512))
                all_chunks.append((phq, i, gstart + off, c))
                off += c
                fill = (fill + c) % 1024
    tiles = []  # each: list of (hq, i, g0, f0, width)
    pos = 0
    while pos < len(all_chunks):
        cur = []
        fill = 0
        while pos < len(all_chunks) and fill < 1024:
            phq, i, g0, c = all_chunks[pos]
            cur.append((phq, i, g0, fill, c))
            fill += c
            pos += 1
        tiles.append(cur)
    stream_by_head = {}  # hq -> [(tile idx, chunk idx)]
    for ti, cur in enumerate(tiles):
        for n, (phq, i, g0, f0, c) in enumerate(cur):
            stream_by_head.setdefault(phq, []).append((ti, n))
    tile_sc = {}
    segs_all = dict(early_segs) if early_segs else {}

    with tc.tile_pool(name=f"a{jj}pr", bufs=1 if prp_ext else 26) as prp0, \
         tc.tile_pool(name=f"a{jj}rc", bufs=2) as rcp, \
         tc.tile_pool(name=f"a{jj}sc", bufs=2 if extra else 3,
                      space="PSUM") as pssc, \
         tc.tile_pool(name=f"a{jj}av", bufs=1, space="PSUM") as psav:
        prp = prp_ext if prp_ext is not None else prp0

        def emit_scores(hq):
            """Scores matmuls + mask + exp for one head; returns probs."""
            hv = hq // 4
            d = hq % 4
            po = hv * 64
            kh = kT[po:po + 64, :]

            def q_ap(a, b):  # global q columns [a, b)
                return qT[po:po + 64, d * 2048 + a: d * 2048 + b]

            if early_prd is not None and hq in early_prd:
                prd = early_prd[hq]
            else:
                # diagonal 128x128 tiles, batched 8-up into one PSUM tile
                dgps = pssc.tile([128, 1024], F32, tag="sc",
                                 name=f"dg{jj}_{hq}")
                for ti in range(8):
                    i = 8 * jj + ti
                    nc.tensor.matmul(
                        dgps[:, ti * 128:(ti + 1) * 128],
                        lhsT=kh[:, i * 128:(i + 1) * 128],
                        rhs=q_ap(q0 + ti * 128, q0 + (ti + 1) * 128),
                        start=(ti % 4 == 0), stop=(ti % 4 == 3))
                nc.vector.scalar_tensor_tensor(
                    out=dgps[:], in0=dgps[:], scalar=0.125, in1=mask_sb[:],
                    op0=ALU.mult, op1=ALU.add)
                prd = prp.tile([128, 1024], BF16, tag="pr",
                               name=f"prd{jj}_{hq}")
                nc.scalar.activation(prd[:], dgps[:], AF.Exp)

            # off-diagonal strips: emit this head's portion of the
            # half-wide packed stream (see plan above); exp fires when a
            # shared [128,1024] tile completes, possibly mid-next-head.
            for ti, n in stream_by_head.get(hq, []):
                t_chunks = tiles[ti]
                if ti not in tile_sc:
                    tile_sc[ti] = pssc.tile([128, 1024], F32, tag="sc",
                                            name=f"sc{jj}_t{ti}")
                sc = tile_sc[ti]
                chq, i, g0, f0, c = t_chunks[n]
                bank = f0 // 512
                first = all(t_chunks[m][3] // 512 != bank for m in range(n))
                last = all(t_chunks[m][3] // 512 != bank
                           for m in range(n + 1, len(t_chunks)))
                kh2 = kT[(chq // 4) * 64:(chq // 4) * 64 + 64, :]
                d2 = chq % 4
                nc.tensor.matmul(
                    sc[:, f0:f0 + c],
                    lhsT=kh2[:, i * 128:(i + 1) * 128],
                    rhs=qT[(chq // 4) * 64:(chq // 4) * 64 + 64,
                           d2 * 2048 + g0: d2 * 2048 + g0 + c],
                    start=first, stop=last)
                if n == len(t_chunks) - 1:
                    fill = f0 + c
                    if fresh_left[0] > 0:
                        fresh_left[0] -= 1
                        prs = fresh_prp.tile([128, 1024], BF16, tag="fpr",
                                             name=f"fprs{jj}_t{ti}")
                    else:
                        prs = prp.tile([128, 1024], BF16, tag="pr",
                                       name=f"prs{jj}_t{ti}")
                    nc.scalar.activation(prs[:, 0:fill], sc[:, 0:fill],
                                         AF.Exp, scale=0.125)
                    for chq2, i2, g2, f2, c2 in t_chunks:
                        for b in range(c2 // 128):
                            segs_all[(chq2, i2, g2 // 128 + b)] = \
                                (prs, f2 + 128 * b)
            return prd, segs_all

        def emit_av(hq, prd, segs):
            """attn@V in natural layout: out[q, f] tiles, probs stationary.

            One [128q, 65] accumulation per q-subtile t (65-row moving cost
            instead of 512), denominator in column 64. Normalized into
            a_buf; the XBAR DMA transpose to aT happens per d-pair later.
            """
            hv = hq // 4
            d = hq % 4

            def v_ap(i):
                return v_aug[:, hv * 1040 + i * 65: hv * 1040 + i * 65 + 65]

            pnat = psav.tile([128, 1024], F32, tag="av",
                             name=f"av{jj}_{hq}")
            mms = []  # (bank, t, i)
            for t in range(8):
                for i in range(8 * jj + t + 1):
                    mms.append((t // 4, t, i))
            first_in_bank = {}
            last_in_bank = {}
            for n, mm in enumerate(mms):
                first_in_bank.setdefault(mm[0], n)
                last_in_bank[mm[0]] = n
            for n, (bank, t, i) in enumerate(mms):
                if i == 8 * jj + t:
                    pr_ap = prd[:, t * 128:(t + 1) * 128]
                else:
                    prs, lc = segs[(hq, i, (q0 + 128 * t) // 128)]
                    pr_ap = prs[:, lc:lc + 128]
                nc.tensor.matmul(pnat[:, t * 128: t * 128 + 65],
                                 lhsT=pr_ap, rhs=v_ap(i),
                                 start=(n == first_in_bank[bank]),
                                 stop=(n == last_in_bank[bank]))

            # normalize into a_buf (bf16): one strided reciprocal + one
            # 3D broadcast multiply for all 8 subtiles
            rc = rcp.tile([128, 8], F32, tag="rc", name=f"rc{jj}_{hq}")
            nc.vector.reciprocal(rc[:], pnat[:, 64::128])
            src3 = pnat[:].rearrange("p (t f) -> p t f", t=8)[:, :, 0:64]
            rcb = rc[:].unsqueeze(2).broadcast_to([128, 8, 64])
            fbase = d * 128 + hv * 64
            dst3 = a_buf[:].rearrange("p (t f) -> p t f", t=8)[
                :, :, fbase:fbase + 64]
            nc.vector.tensor_mul(dst3, src3, rcb)

        def emit_transpose(d):
            """aT[d-block] <- XBAR transpose of a_buf columns (both hv)."""
            for t in range(8):
                nc.sync.dma_start(
                    out=aT[0:128, d * 2048 + q0 + t * 128:
                           d * 2048 + q0 + (t + 1) * 128],
                    in_=a_buf[:, t * 512 + d * 128: t * 512 + (d + 1) * 128],
                    transpose=True)

        # cross-head pipeline: head h+1's scores run on the PE while the
        # ACT engine exponentiates head h; ACT never waits behind attn@V.
        pending = None
        navs = 0
        for hq in range(8):
            scores = emit_scores(hq)
            if pending is not None:
                emit_av(*pending)
                if pending[0] >= 4:
                    emit_transpose(pending[0] - 4)
                navs += 1
                if extra and navs >= 3:
                    extra.pop(0)()
            pending = (hq, *scores)
        emit_av(*pending)
        emit_transpose(pending[0] - 4)
        while extra:
            extra.pop(0)()


def _load_wo(tc, wop, woT):
    """Prefetch all Wo tiles on the idle SP queue (during attention)."""
    nc = tc.nc
    wo_tiles = {}
    for cb in range(4):
        for f in range(4):
            wt = wop.tile([128, 512], BF16, tag="wo", name=f"wo{cb}_{f}")
            nc.gpsimd.dma_start(
                out=wt[:],
                in_=woT[f * 128:(f + 1) * 128, cb * 512:(cb + 1) * 512])
            wo_tiles[(cb, f)] = wt
    return wo_tiles


def _phase3_cb(tc, wo_all, aT, out, ostp, psop, cb, tts, act_copy=False,
               swdge_store=False):
    """Output-projection tiles for one 512-col block of Wo, given t-tiles."""
    nc = tc.nc
    wo_tiles = [wo_all[(cb, f)] for f in range(4)]
    for tt in tts:
        pop_ = psop.tile([128, 512], F32, tag="op", name=f"op{cb}_{tt}")
        for f in range(4):
            a_ap = aT[:, f * 2048 + tt * 128: f * 2048 + tt * 128 + 128]
            nc.tensor.matmul(pop_[:], lhsT=a_ap, rhs=wo_tiles[f][:],
                             start=(f == 0), stop=(f == 3))
        ost = ostp.tile([128, 512], F32, tag="ost", name=f"ost{cb}_{tt}")
        if act_copy and tt % 2 == 0:
            nc.scalar.copy(ost[:], pop_[:])
        else:
            nc.vector.tensor_copy(ost[:], pop_[:])
        if swdge_store:
            nc.gpsimd.dma_start(
                out=out[tt * 128:(tt + 1) * 128, cb * 512:(cb + 1) * 512],
                in_=ost[:])
        else:
            nc.sync.dma_start(
                out=out[tt * 128:(tt + 1) * 128, cb * 512:(cb + 1) * 512],
                in_=ost[:])


def _body(tc, io):
    nc = tc.nc
    xT, wqT, wkvT, woT, cosT, sinT, maskT, permT, out = io

    with tc.tile_pool(name="const", bufs=1) as cst:
        wq = cst.tile([128, 16 * 512], BF16, name="wq")
        wkv = cst.tile([128, 16 * 256], BF16, name="wkv")
        cos_sb = cst.tile([128, T], BF16, name="cos_sb")
        sin_sb = cst.tile([128, T], BF16, name="sin_sb")
        mask_sb = cst.tile([128, 1024], F32, name="mask_sb")
        perm_f32 = cst.tile([128, 128], F32, name="perm_f32")
        perm_sb = cst.tile([128, 128], BF16, name="perm_sb")
        ident = cst.tile([128, 128], BF16, name="ident")
        kT = cst.tile([128, T], BF16, name="kT")
        v_aug = cst.tile([128, 2 * 16 * 65], BF16, name="v_aug")
        qT = cst.tile([128, 4 * 2048], BF16, name="qT")
        aT = cst.tile([128, 4 * 2048], BF16, name="aT")

        for c in range(16):
            nc.gpsimd.dma_start(out=wkv[:, c * 256:(c + 1) * 256],
                                in_=wkvT[c * 128:(c + 1) * 128, :])
        nc.gpsimd.dma_start(out=cos_sb[:], in_=cosT[:])
        nc.gpsimd.dma_start(out=sin_sb[:], in_=sinT[:])
        nc.gpsimd.dma_start(out=mask_sb[:], in_=maskT[:])
        nc.gpsimd.dma_start(out=perm_f32[:], in_=permT[:])
        nc.vector.tensor_copy(perm_sb[:], perm_f32[:])
        make_identity(nc, ident[:])
        # ones columns of v_aug come from this memset (data written over it)
        nc.vector.memset(v_aug[:], 1.0)
        warm = cst.tile([1, 8], BF16, name="warm")
        nc.scalar.activation(warm[:], mask_sb[0:1, 0:8], AF.Exp)

        p1c = (xT, wqT, wq, wkv, cos_sb, sin_sb, perm_sb, ident, qT, kT,
               v_aug)
        attc = (qT, kT, v_aug, mask_sb, aT)

        early_prd = {}
        early_segs0 = {}

        with tc.tile_pool(name="p3wo", bufs=16) as wop, \
             tc.tile_pool(name="abq", bufs=2) as abq, \
             tc.tile_pool(name="a0pre", bufs=10) as a0pre, \
             tc.tile_pool(name="edpr", bufs=8) as eprp:
            fpr = None
            # natural-layout attn outputs for each half: subtile t at cols
            # t*512, feature f = d*128 + hv*64 + dh (matches aT partition
            # order after the XBAR transpose)
            a_buf0 = abq.tile([128, 8 * 512], BF16, tag="ab", name="ab0")
            a_buf1 = abq.tile([128, 8 * 512], BF16, tag="ab", name="ab1")
            def emit_early_diag(jj, dst):
                q0 = jj * 1024
                with tc.tile_pool(name=f"edps{jj}", bufs=2,
                                  space="PSUM") as edps:
                    for hq in range(8):
                        hv = hq // 4
                        d = hq % 4
                        po = hv * 64
                        prd = eprp.tile([128, 1024], BF16, tag="epr",
                                        name=f"eprd{jj}_{hq}")
                        for half in range(2):
                            dg = edps.tile([128, 512], F32, tag="ed",
                                           name=f"edg{jj}_{hq}_{half}")
                            for tl in range(4):
                                ti = half * 4 + tl
                                i = 8 * jj + ti
                                nc.tensor.matmul(
                                    dg[:, tl * 128:(tl + 1) * 128],
                                    lhsT=kT[po:po + 64,
                                            i * 128:(i + 1) * 128],
                                    rhs=qT[po:po + 64,
                                           d * 2048 + q0 + ti * 128:
                                           d * 2048 + q0 + (ti + 1) * 128],
                                    start=(tl == 0), stop=(tl == 3))
                            nc.vector.scalar_tensor_tensor(
                                out=dg[:], in0=dg[:], scalar=0.125,
                                in1=mask_sb[:, 0:512],
                                op0=ALU.mult, op1=ALU.add)
                            nc.scalar.activation(
                                prd[:, half * 512:(half + 1) * 512],
                                dg[:], AF.Exp)
                        dst[hq] = prd
                    if jj == 0 and hq == 0:
                        # head 0's off-diagonal strips, packed 512 at a
                        # time through the same early PSUM pool
                        chunks = []
                        for i in range(8):
                            gstart = 128 * (i + 1)
                            W = 1024 - gstart
                            off = 0
                            fill = sum(ch[2] for ch in chunks) % 512
                            while off < W:
                                c = min(W - off, 512 - fill)
                                chunks.append((i, gstart + off, c))
                                off += c
                                fill = (fill + c) % 512
                        pos = 0
                        nt = 0
                        while pos < len(chunks):
                            cur = []
                            fill = 0
                            while pos < len(chunks) and fill < 512:
                                i, g0, c = chunks[pos]
                                cur.append((i, g0, fill, c))
                                fill += c
                                pos += 1
                            es = edps.tile([128, 512], F32, tag="ed",
                                           name=f"es{nt}")
                            for n, (i, g0, f0, c) in enumerate(cur):
                                nc.tensor.matmul(
                                    es[:, f0:f0 + c],
                                    lhsT=kT[0:64, i * 128:(i + 1) * 128],
                                    rhs=qT[0:64, g0:g0 + c],
                                    start=(n == 0), stop=(n == len(cur) - 1))
                            ep = eprp.tile([128, 1024], BF16, tag="epr",
                                           name=f"esp{nt // 2}") \
                                if nt % 2 == 0 else last_ep
                            last_ep = ep
                            half2 = (nt % 2) * 512
                            nc.scalar.activation(
                                ep[:, half2:half2 + fill],
                                es[:, 0:fill], AF.Exp, scale=0.125)
                            for i, g0, f0, c in cur:
                                for b in range(c // 128):
                                    early_segs0[(0, i, g0 // 128 + b)] = \
                                        (ep, half2 + f0 + 128 * b)
                            nt += 1

            _phase1(tc, cst, p1c,
                    after_j1=lambda: emit_early_diag(0, early_prd))
            _attn_half(tc, 0, attc, a_buf0, early_prd=early_prd,
                       prp_ext=a0pre, early_segs=early_segs0)
            wo_all = _load_wo(tc, wop, woT)
            early_prd1 = {}
            emit_early_diag(1, early_prd1)
            with tc.tile_pool(name="p3ost", bufs=4) as ostp, \
                 tc.tile_pool(name="p3ps", bufs=2, space="PSUM") as psop:
                # first half of the output projection (q < 1024, complete
                # after attention half 0) rides inside the ACT-bound half 1
                extra = [
                    (lambda cb=cb: _phase3_cb(tc, wo_all, aT, out, ostp,
                                              psop, cb, range(8),
                                              swdge_store=True))
                    for cb in range(4)
                ]
                _attn_half(tc, 1, attc, a_buf1, extra_work=extra,
                           early_prd=early_prd1)
                for cb in range(4):
                    _phase3_cb(tc, wo_all, aT, out, ostp, psop, cb,
                               range(8, 16), act_copy=True)


_cached_nc = None


def _build():
    global _cached_nc
    if _cached_nc is not None:
        return _cached_nc
    nc = bacc.Bacc("TRN2", target_bir_lowering=False, debug=False,
                   num_devices=NCORE)
    io = (
        nc.dram_tensor("xT", [C, T], BF16, kind="ExternalInput").ap(),
        nc.dram_tensor("wqT", [C, 512], BF16, kind="ExternalInput").ap(),
        nc.dram_tensor("wkvT", [C, 256], BF16, kind="ExternalInput").ap(),
        nc.dram_tensor("woT", [512, C], BF16, kind="ExternalInput").ap(),
        nc.dram_tensor("cosT", [128, T], BF16, kind="ExternalInput").ap(),
        nc.dram_tensor("sinT", [128, T], BF16, kind="ExternalInput").ap(),
        nc.dram_tensor("maskT", [128, 1024], F32, kind="ExternalInput").ap(),
        nc.dram_tensor("permT", [128, 128], F32, kind="ExternalInput").ap(),
        nc.dram_tensor("out", [T, C], F32, kind="ExternalOutput").ap(),
    )
    with tile.TileContext(nc) as tc:
        with nc.allow_low_precision(reason="bf16/fp32r matmul operands"):
            _body(tc, io)
    nc.compile()
    _cached_nc = nc
    return nc


def _prep_in_maps(x, cos, sin, Wq, Wkv, Wo):
    x = np.asarray(x, np.float32)
    cos = np.asarray(cos, np.float32)
    sin = np.asarray(sin, np.float32)
    Wq = np.asarray(Wq, np.float32)
    Wkv = np.asarray(Wkv, np.float32)
    Wo = np.asarray(Wo, np.float32)

    p = np.arange(128)
    cosT = np.ascontiguousarray(cos[:, p % 32].T).astype(ml_dtypes.bfloat16)
    sinT = np.ascontiguousarray(sin[:, p % 32].T).astype(ml_dtypes.bfloat16)

    # one diagonal 128x128 causal mask (0 where k<=q, -1e30 above), tiled x8
    n = np.arange(128)
    m1 = np.where(p[:, None] <= n[None, :], 0.0, NEG).astype(np.float32)
    maskT = np.ascontiguousarray(np.tile(m1, (1, 8)))

    # signed rotate-half permutation: sh[m] = -raw[m+32] (m%64<32)
    #                                  sh[m] = +raw[m-32] (m%64>=32)
    permT = np.zeros((128, 128), np.float32)
    for m in range(128):
        base = (m // 64) * 64
        r = m % 64
        if r < 32:
            permT[base + r + 32, m] = -1.0
        else:
            permT[base + r - 32, m] = 1.0

    in_maps = []
    for b in range(B):
        xTb = np.ascontiguousarray(x[b].T).astype(ml_dtypes.bfloat16)
        for g in range(4):
            perm = np.empty(512, np.int64)
            for dd_t in range(4):
                for o in (0, 64):
                    hq = dd_t + (o // 64) * 4
                    perm[dd_t * 128 + o: dd_t * 128 + o + 64] = \
                        np.arange(hq * 64, hq * 64 + 64)
            wqT = np.ascontiguousarray(Wq[g * 512:(g + 1) * 512, :][perm].T).astype(ml_dtypes.bfloat16)
            wkvT = np.ascontiguousarray(np.concatenate(
                [Wkv[128 * g:128 * g + 128],
                 Wkv[512 + 128 * g:512 + 128 * g + 128]], 0).T).astype(
                ml_dtypes.bfloat16)
            woT = np.ascontiguousarray(
                Wo[:, g * 512:(g + 1) * 512].T[perm]).astype(ml_dtypes.bfloat16)
            in_maps.append({"xT": xTb, "wqT": wqT, "wkvT": wkvT, "woT": woT,
                            "cosT": cosT, "sinT": sinT, "maskT": maskT,
                            "permT": permT})
    return in_maps


def _run(x, cos, sin, Wq, Wkv, Wo, trace=False):
    nc = _build()
    in_maps = _prep_in_maps(x, cos, sin, Wq, Wkv, Wo)
    res = bass_utils.run_bass_kernel_spmd(nc, in_maps,
                                          core_ids=list(range(NCORE)),
                                          trace=trace)
    out = np.zeros((B, T, C), np.float32)
    for b in range(B):
        for g in range(4):
            out[b] += res.results[b * 4 + g]["out"]
    return out, res


def kernel(x, cos, sin, Wq, Wkv, Wo):
    out, _ = _run(x, cos, sin, Wq, Wkv, Wo)
    return out
